# revision 1
# baseline (speedup 1.0000x reference)
"""Spatial-softmax expectation kernel for Trainium2, 8-core SPMD.

Computes, for x of shape [B=32, C=128, H=128, W=128]:
    prob = softmax(x.reshape(B, C, H*W), axis=-1)
    keypoints[b, c] = (sum_n prob[n] * xs[w(n)], sum_n prob[n] * ys[h(n)])

Strategy (per (b, c) row the result factorizes through marginals):
    e       = exp(x - 2)                    (softmax is shift-invariant)
    marg_h  = sum_w e                       -> num_y = sum_h ys * marg_h
    marg_w  = sum_h e                       -> num_x = sum_w xs * marg_w
    s       = sum marg_h
    k       = (num_x / s, num_y / s)

Mapping: rows (b, c) on SBUF partitions, H*W on the free dim, chunked.
  - ScalarE: exp (fp32 -> fp16 output; only e is quantized, sums stay fp32)
  - VectorE: marg_h via free-axis reduce over W
  - TensorE: marg_w via fp16 identity-matmuls accumulating in fp32 PSUM
  - DMA:     ~32 MiB/core streaming, the roofline term.  Input loads
    alternate between the two HWDGE queues (qSP / qAct) so per-DMA
    descriptor-gen + DGE-delay overheads on one queue hide under the
    other queue's transfer; stores go to the gpsimd SWDGE queue.
Sharding: data-parallel over B*C rows; core i takes rows [i*512, (i+1)*512).
"""

import numpy as np

import concourse.bacc as bacc
import concourse.mybir as mybir
import concourse.tile as tile
from concourse.bass_utils import run_bass_kernel_spmd

B, C, H, W = 32, 128, 128, 128
N_CORES = 8
ROWS = B * C                    # 4096 (b, c) rows total
ROWS_PER_CORE = ROWS // N_CORES  # 512
HW = H * W                       # 16384
GROUP = 128                      # rows per partition-group
EXP_BIAS = -2.0                  # exp(x-2): fp16 range safety, cancels in ratio

FP32 = mybir.dt.float32
FP16 = mybir.dt.float16

DEFAULT_CHUNK = 2048
DEFAULT_QUEUES = ("sync",)
DEFAULT_TAIL_SPLIT = 0
DEFAULT_STORE_ENG = "gpsimd"

_DBUFS = {512: 12, 1024: 10, 2048: 6, 4096: 4, 8192: 3}

_cache = {}


def build_program(
    rows=ROWS_PER_CORE,
    chunk=DEFAULT_CHUNK,
    n_reps=1,
    loop_n=None,
    queues=DEFAULT_QUEUES,
    tail_split=DEFAULT_TAIL_SPLIT,
    store_eng=DEFAULT_STORE_ENG,
    dbufs=None,
    mbufs=2,
    pbufs=2,
    sbufs=2,
    stages=frozenset({"act", "dve", "pe"}),
):
    """Build and compile the single-core Bass program (run SPMD on 8 cores).

    n_reps > 1 statically repeats the computation in-program; loop_n wraps it
    in a hardware For_i loop (both for benchmarking: per-rep HW time is the
    slope of wall time vs rep count, launch overhead cancels).
    """
    n_groups = rows // GROUP
    n_chunks = HW // chunk
    if dbufs is None:
        dbufs = _DBUFS[chunk]

    nc = bacc.Bacc("TRN2", target_bir_lowering=False, debug=False)

    x_d = nc.dram_tensor("x", [rows, HW], FP32, kind="ExternalInput")
    xsb_d = nc.dram_tensor("xsb", [128, W], FP32, kind="ExternalInput")
    ysb_d = nc.dram_tensor("ysb", [128, H], FP32, kind="ExternalInput")
    idf_d = nc.dram_tensor("idf", [128, 128], FP16, kind="ExternalInput")
    y_d = nc.dram_tensor("y", [rows, 2], FP32, kind="ExternalOutput")

    with tile.TileContext(nc) as tc:
        with (
            tc.tile_pool(name="const", bufs=1) as cpool,
            tc.tile_pool(name="data", bufs=dbufs) as dpool,
            tc.tile_pool(name="marg", bufs=mbufs) as mpool,
            tc.tile_pool(name="small", bufs=sbufs) as spool,
            tc.tile_pool(name="psum", bufs=pbufs, space="PSUM") as ppool,
        ):
            xsb = cpool.tile([128, W], FP32, tag="xsb")
            ysb = cpool.tile([128, H], FP32, tag="ysb")
            idf = cpool.tile([128, 128], FP16, tag="idf")
            bias_t = cpool.tile([128, 1], FP32, tag="bias")
            nc.sync.dma_start(xsb[:], xsb_d[:, :])
            nc.sync.dma_start(ysb[:], ysb_d[:, :])
            nc.sync.dma_start(idf[:], idf_d[:, :])
            nc.vector.memset(bias_t[:], EXP_BIAS)
            dummy_out = None
            if "dve" not in stages and "pe" not in stages:
                dummy_out = cpool.tile([128, 2], FP32, tag="dummy")
                nc.vector.memset(dummy_out[:], 0.0)

            load_engines = [getattr(nc, q) for q in queues]
            store_engine = getattr(nc, store_eng)
            dma_counter = [0]

            def emit_group(g, last_group):
                rows_lo = g * GROUP
                margw_ps = ppool.tile([128, W], FP32, tag="margw")
                margh = mpool.tile([128, H], FP32, tag="margh")

                # Optionally split the final chunk of the last group into fine
                # sub-chunks (shortens the exposed one-shot tail; irrelevant
                # for steady-state loop throughput).
                sizes = [chunk] * n_chunks
                if last_group and tail_split and chunk >= 2 * tail_split:
                    sizes = sizes[:-1] + [tail_split] * (chunk // tail_split)
                assert sum(sizes) == HW

                off = 0
                for size in sizes:
                    h_per = size // W
                    h_base = off // W
                    xt = dpool.tile([128, size], FP32, tag=f"xt{size}")
                    eng = load_engines[dma_counter[0] % len(load_engines)]
                    dma_counter[0] += 1
                    eng.dma_start(
                        xt[:], x_d[rows_lo : rows_lo + GROUP, off : off + size]
                    )
                    if "act" not in stages:
                        off += size
                        continue
                    et = dpool.tile([128, size], FP16, tag=f"et{size}")
                    nc.scalar.activation(
                        et[:], xt[:], mybir.ActivationFunctionType.Exp, bias=bias_t[:]
                    )
                    e3 = et[:].rearrange("p (h w) -> p h w", w=W)
                    if "dve" in stages:
                        nc.vector.reduce_sum(
                            margh[:, h_base : h_base + h_per],
                            e3,
                            axis=mybir.AxisListType.X,
                        )
                    if "pe" in stages:
                        for hh in range(h_per):
                            nc.tensor.matmul(
                                margw_ps[:],
                                idf[:],
                                e3[:, hh, :],
                                start=(off == 0 and hh == 0),
                                stop=(off + size == HW and hh == h_per - 1),
                            )
                    off += size

                if "dve" not in stages and "pe" not in stages:
                    store_engine.dma_start(
                        y_d[rows_lo : rows_lo + GROUP, :], dummy_out[:]
                    )
                    return
                margw_src = margw_ps if "pe" in stages else margh
                margh_src = margh if "dve" in stages else margw_ps
                # tensor_tensor_reduce would fuse these, but that opcode
                # hard-faults the exec unit on this runtime; use mul+reduce.
                scr_x = spool.tile([128, W], FP32, tag="scrx")
                scr_y = spool.tile([128, H], FP32, tag="scry")
                num_xy = spool.tile([128, 2], FP32, tag="numxy")
                nc.vector.tensor_mul(scr_x[:], margw_src[:], xsb[:])
                nc.vector.reduce_sum(num_xy[:, 0:1], scr_x[:], axis=mybir.AxisListType.X)
                nc.vector.tensor_mul(scr_y[:], margh_src[:], ysb[:])
                nc.vector.reduce_sum(num_xy[:, 1:2], scr_y[:], axis=mybir.AxisListType.X)
                s = spool.tile([128, 1], FP32, tag="s")
                nc.vector.reduce_sum(s[:], margh_src[:], axis=mybir.AxisListType.X)
                recip = spool.tile([128, 1], FP32, tag="recip")
                nc.vector.reciprocal(recip[:], s[:])
                out_t = spool.tile([128, 2], FP32, tag="out")
                nc.vector.tensor_scalar_mul(out_t[:], num_xy[:], recip[:])
                store_engine.dma_start(y_d[rows_lo : rows_lo + GROUP, :], out_t[:])

            def emit_all():
                for _rep in range(n_reps):
                    for g in range(n_groups):
                        emit_group(g, last_group=(g == n_groups - 1))

            if loop_n is not None:
                with tc.For_i(0, loop_n, 1, hint_engines=(mybir.EngineType.PE,)):
                    emit_all()
            else:
                emit_all()

    nc.compile()
    return nc


def make_consts():
    xs = np.linspace(-1.0, 1.0, W).astype(np.float32)
    ys = np.linspace(-1.0, 1.0, H).astype(np.float32)
    return {
        "xsb": np.ascontiguousarray(np.tile(xs, (128, 1))),
        "ysb": np.ascontiguousarray(np.tile(ys, (128, 1))),
        "idf": np.eye(128, dtype=np.float16),
    }


def kernel(x):
    x = np.ascontiguousarray(np.asarray(x), dtype=np.float32)
    assert x.shape == (B, C, H, W), x.shape

    if "nc" not in _cache:
        _cache["nc"] = build_program()
    nc = _cache["nc"]

    consts = make_consts()
    xf = x.reshape(N_CORES, ROWS_PER_CORE, HW)
    in_maps = [{"x": xf[i], **consts} for i in range(N_CORES)]
    res = run_bass_kernel_spmd(nc, in_maps, list(range(N_CORES))).results
    y = np.stack([res[i]["y"] for i in range(N_CORES)], axis=0)  # [8, 512, 2]
    return y.reshape(B, C, 2)

